# revision 12
# baseline (speedup 1.0000x reference)
"""Multi-head causal self-attention on 8 Trainium2 NeuronCores.

Sharding: core c -> batch b = c // 2, heads 4*(c % 2) .. +4  (data parallel on
B, tensor parallel on heads).  Each core computes its 4 heads' attention for
its batch plus the partial out-projection; the host sums the two partials per
batch and adds b_out.

Per-core layout:
  xT   [D, T]      x[b] transposed on host (bf16)
  qT/kT [128, 2, T] head-major: partitions = 2 heads x 64, 2 m-tiles
  v    [128, 16, 260] natural [T, hd] per head + a ones column (gives the
                    softmax denominator for free during the AV matmul)
  scores are computed transposed: sT[k, q] = kT.T @ q, both heads of a pair
  packed into one [128, 2, 512] PSUM tile so a single evacuation call
  covers both; exp'd during PSUM->SBUF evacuation (no max subtraction:
  |scores| <= ~3 here), causality via tile skipping/trimming + one
  upper-triangular 128x128 mask constant.

Attention runs in 512-query units (qb, head-pair), query-block outer.  The
PE instruction stream is software-pipelined: AV matmuls lag the score
matmuls by one key-tile step (the PE queue is strictly in-order, so
without the lag every AV would stall on its own tile's exp evacuation),
AV accumulator PSUM sets alternate per unit, and each query block's
out-projection tiles are interleaved into the NEXT query block's score
stream as boundary filler.  The exp evacuation alternates by key-tile
parity between ACT (exact spline exp) and DVE (Schraudolph bit-trick exp2
via tensor_scalar -> int16 -> bf16 bitcast, ~3% per element, cancels in
softmax); triangular masks and normalization multiplies are split between
DVE and GpSimd.  The out-projection packs head pairs into K=128
accumulations (odd heads hop partitions via SBUF-to-SBUF DMA).
"""

import os
import sys
from contextlib import ExitStack

import numpy as np

for _p in ("/opt/trn_rl_repo", "/opt/pypackages"):
    if os.path.isdir(_p) and _p not in sys.path:
        sys.path.append(_p)

import concourse.bass as bass
from concourse import bacc
import concourse.mybir as mybir
import concourse.tile as tile
from concourse.bass_utils import run_bass_kernel_spmd
from concourse.masks import make_upper_triangular


B, T, D = 4, 2048, 512
H, HD = 8, 64
HPC = 4  # heads per core
P = 128
KT = D // P  # k-tiles over the model dim
QB = 512  # query-unit width / psum bank width
NKT = T // P  # key tiles
NU = T // QB  # query blocks
VW = HD + 1  # v columns per head incl. the ones column

F32 = mybir.dt.float32
I16 = mybir.dt.int16
BF16 = mybir.dt.bfloat16
MMDT = BF16
EXP = mybir.ActivationFunctionType.Exp

# fast-exp constants: exp(s/8) ~= bitcast_bf16(int16(s * FE_A + FE_B))
_LOG2E = 1.4426950408889634
FE_A = _LOG2E * 128.0 / 8.0
FE_B = 127.0 * 128.0 - 5.6

try:
    import ml_dtypes
    _NP_MMDT = np.dtype(ml_dtypes.bfloat16)
except ImportError:
    _NP_MMDT = np.float32


def build_bass():
    nc = bacc.Bacc()
    xT = nc.declare_dram_parameter("xT", [D, T], MMDT, isOutput=False)
    wqa = nc.declare_dram_parameter("wqa", [P, KT, 2 * P], MMDT, isOutput=False)
    wka = nc.declare_dram_parameter("wka", [P, KT, 2 * P], MMDT, isOutput=False)
    # q/k biases, laid out [channel % 128, channel // 128] for ACT bias APs
    wqkb = nc.declare_dram_parameter("wqkb", [P, 4], F32, isOutput=False)
    wva = nc.declare_dram_parameter("wva", [P, KT, HPC * VW], MMDT, isOutput=False)
    wo = nc.declare_dram_parameter("wo", [P, 2, D], MMDT, isOutput=False)
    y = nc.declare_dram_parameter("y", [T, D], BF16, isOutput=True)

    with tile.TileContext(nc) as tc, ExitStack() as ctx:
        consts = ctx.enter_context(tc.tile_pool(name="consts", bufs=1))
        qkv = ctx.enter_context(tc.tile_pool(name="qkv", bufs=1))
        attn = ctx.enter_context(tc.tile_pool(name="attn", bufs=1))
        etp = ctx.enter_context(tc.tile_pool(name="etp", bufs=4))
        nrm = ctx.enter_context(tc.tile_pool(name="nrm", bufs=4))
        yevac = ctx.enter_context(tc.tile_pool(name="yevac", bufs=3))
        # PSUM: "mm" 2 bufs x 4KB/partition (2 banks each) = 4 banks;
        # o{i}{s} 4 x [128,512]f32 (1 bank each) = 4 banks.  AV accumulator
        # sets s alternate per unit; QKV + out-proj borrow "mm".
        mmps = ctx.enter_context(tc.tile_pool(name="mmps", bufs=2, space="PSUM"))
        aps = ctx.enter_context(tc.tile_pool(name="aps", bufs=1, space="PSUM"))

        # ---- inputs / constants into SBUF (ordered for early compute start;
        # issue spread across the three DMA-capable engine queues)
        x_sb = consts.tile([P, KT, T], MMDT)
        wq_sb = consts.tile([P, KT, 2 * P], MMDT)
        wk_sb = consts.tile([P, KT, 2 * P], MMDT)
        wv_sb = consts.tile([P, KT, HPC * VW], MMDT)
        wqkb_sb = consts.tile([P, 4], F32)
        wo_sb = consts.tile([P, 2, D], MMDT)

        dmae = (nc.sync, nc.scalar, nc.gpsimd)
        nc.sync.dma_start(out=wq_sb, in_=wqa[:])
        nc.scalar.dma_start(out=wk_sb, in_=wka[:])
        n = 0
        for ch in range(4):  # 512-col chunks so the first matmul starts early
            for kt in range(KT):
                dmae[n % 3].dma_start(
                    out=x_sb[:, kt, ch * QB : (ch + 1) * QB],
                    in_=xT[kt * P : (kt + 1) * P, ch * QB : (ch + 1) * QB],
                )
                n += 1
            if ch == 0:
                nc.gpsimd.dma_start(out=wqkb_sb, in_=wqkb[:])
            if ch == 1:
                nc.scalar.dma_start(out=wv_sb, in_=wva[:])
        nc.gpsimd.dma_start(out=wo_sb, in_=wo[:])

        # triu[k, q] = 1 iff q >= k: allowed region of a diagonal block in
        # transposed-score space.  gpsimd affine_select needs f32; cast after.
        triu_st = consts.tile([P, P], F32)
        make_upper_triangular(nc, triu_st, val=1.0, diag=True)
        triu = consts.tile([P, P], MMDT)
        nc.vector.tensor_copy(triu, triu_st)

        # ---- QKV projections
        qT_sb = qkv.tile([P, 2, T], MMDT)
        kT_sb = qkv.tile([P, 2, T], MMDT)
        v_sb = qkv.tile([P, NKT, HPC * VW], MMDT)

        WB = 1024  # bf16 moving-operand max

        def qk_proj(wi, w_sb, dst, m, nb):
            ps = mmps.tile([P, WB], F32, tag="mm", name="ps")
            for lo in range(0, WB, QB):  # psum-bank-sized writes
                for kt in range(KT):
                    nc.tensor.matmul(
                        ps[:, lo : lo + QB],
                        lhsT=w_sb[:, kt, m * P : (m + 1) * P],
                        rhs=x_sb[:, kt, nb * WB + lo : nb * WB + lo + QB],
                        start=(kt == 0),
                        stop=(kt == KT - 1),
                    )
            nc.scalar.activation(
                out=dst[:, m, nb * WB : (nb + 1) * WB], in_=ps,
                func=mybir.ActivationFunctionType.Identity,
                bias=wqkb_sb[:, 2 * wi + m : 2 * wi + m + 1],
            )

        # v bias is folded into the host-side output bias (b_v @ W_out adds a
        # constant row after softmax-normalize + out-projection), so v here is
        # bias-free; the denominator ones-columns are memset directly.
        def v_proj(tt):
            tag = f"o{tt % 2}{'ab'[(tt // 2) % 2]}"
            ps = aps.tile([P, QB], F32, tag=tag, name="vps")
            for kt in range(KT):
                nc.tensor.matmul(
                    ps[:, 0 : HPC * VW],
                    lhsT=x_sb[:, kt, tt * P : (tt + 1) * P],
                    rhs=wv_sb[:, kt, :],
                    start=(kt == 0),
                    stop=(kt == KT - 1),
                )
            nc.scalar.copy(v_sb[:, tt, :], ps[:, 0 : HPC * VW])
            ones_cols = v_sb[:, tt, :].rearrange("p (h w) -> p h w", w=VW)[:, :, HD]
            nc.gpsimd.memset(ones_cols, 1.0)

        # issue order: everything the first attention units need first
        for m in range(2):
            qk_proj(0, wq_sb, qT_sb, m, 0)
            qk_proj(1, wk_sb, kT_sb, m, 0)
        for tt in range(NKT // 2):
            v_proj(tt)
        for m in range(2):
            qk_proj(0, wq_sb, qT_sb, m, 1)
            qk_proj(1, wk_sb, kT_sb, m, 1)
        for tt in range(NKT // 2, NKT):
            v_proj(tt)

        # ---- attention in 512-query units, PE-stream software-pipelined
        # attn pair tiles: rows 0-63 = even head, 64-127 = odd head of pair
        attn_p = [
            attn.tile([P, T], MMDT, tag=f"attnp{hp}", name=f"attnp{hp}")
            for hp in range(2)
        ]

        def emit_norm(hp, qb, opss):
            # normalize: row HD of ops is the softmax denominator.  The
            # copy evacuates the AV psum immediately (frees the bank); the
            # rest of the chain runs off SBUF.
            cols = slice(qb * QB, (qb + 1) * QB)
            for i in (0, 1):
                att_sb = nrm.tile([VW, QB], F32, tag="att", name="att")
                if i == 0:
                    nc.vector.tensor_copy(att_sb, opss[i][0:VW, :])
                else:
                    nc.scalar.copy(att_sb, opss[i][0:VW, :])
                rec = nrm.tile([VW, QB], F32, tag="rec", name="rec")
                nc.vector.reciprocal_approx_fast(out=rec, in_=att_sb)
                den0 = nrm.tile([1, QB], F32, tag="den0", name="den0")
                nc.sync.dma_start(out=den0, in_=rec[HD : HD + 1, :])
                bc = nrm.tile([HD, QB], F32, tag="bc", name="bc")
                nc.gpsimd.partition_broadcast(bc, den0)
                if i == 0:
                    nc.vector.tensor_mul(
                        attn_p[hp][0:HD, cols], att_sb[0:HD, :], bc
                    )
                else:
                    # odd head: normalize into a scratch at lanes 0-63,
                    # then DMA-hop to lanes 64-127 of the pair tile
                    odd = nrm.tile([HD, QB], MMDT, tag="odd", name="odd")
                    nc.gpsimd.tensor_mul(odd, att_sb[0:HD, :], bc)
                    nc.sync.dma_start(out=attn_p[hp][HD:P, cols], in_=odd)

        def emit_outproj_tt(tt):
            # y[t, d] = attn_p0.T @ wo_p0 + attn_p1.T @ wo_p1  (K=128 each)
            ps = mmps.tile([P, D], F32, tag="mm", name="yps")
            for hp in range(2):
                nc.tensor.matmul(
                    ps,
                    lhsT=attn_p[hp][:, tt * P : (tt + 1) * P],
                    rhs=wo_sb[:, hp, :],
                    start=(hp == 0),
                    stop=(hp == 1),
                )
            yt = yevac.tile([P, D], BF16, tag="yt", name="yt")
            if tt % 2:
                nc.scalar.copy(yt, ps)
            else:
                nc.vector.tensor_copy(yt, ps)
            nc.sync.dma_start(out=y[tt * P : (tt + 1) * P, :], in_=yt)

        units = [(qb, hp) for qb in range(NU) for hp in range(HPC // 2)]
        pending = None  # (av_emitter, is_last_kt, norm_emitter)
        for u, (qb, hp) in enumerate(units):
            pair = (2 * hp, 2 * hp + 1)
            qhs = [
                qT_sb[(h % 2) * HD : (h % 2) * HD + HD, h // 2, :] for h in pair
            ]
            khs = [
                kT_sb[(h % 2) * HD : (h % 2) * HD + HD, h // 2, :] for h in pair
            ]
            st = "ab"[u % 2]
            opss = [
                aps.tile([P, QB], F32, tag=f"o{i}{st}", name=f"o{i}{st}")
                for i in range(2)
            ]
            nkt = (qb + 1) * (QB // P)
            # interleave the PREVIOUS query block's out-projection tiles into
            # this unit's score stream (hp==0 units only, late enough that
            # the previous block's normalization chain has drained)
            op_tts = (
                list(range((qb - 1) * (QB // P), qb * (QB // P)))
                if hp == 0 and qb >= 1
                else []
            )
            for kt in range(nkt):
                off = max(0, kt * P - qb * QB)
                # scores for both heads into one [128, 2, 512] psum tile
                sps = mmps.tile([P, 2, QB], F32, tag="mm", name="sps")
                for i in (0, 1):
                    nc.tensor.matmul(
                        sps[:, i, off:QB],
                        lhsT=khs[i][:, kt * P : (kt + 1) * P],
                        rhs=qhs[i][:, qb * QB + off : (qb + 1) * QB],
                        start=True,
                        stop=True,
                    )
                eT = etp.tile([P, 2, QB], MMDT, tag="eT", name="eT")
                if kt % 2:  # DVE fast-exp on odd key tiles
                    nc.vector.tensor_scalar(
                        out=eT[:, :, off:QB].bitcast(I16),
                        in0=sps[:, :, off:QB],
                        scalar1=FE_A, scalar2=FE_B,
                        op0=mybir.AluOpType.mult,
                        op1=mybir.AluOpType.add,
                    )
                else:
                    nc.scalar.activation(
                        out=eT[:, :, off:QB], in_=sps[:, :, off:QB],
                        func=EXP, scale=1.0 / np.sqrt(HD),
                    )
                if kt * P >= qb * QB:  # diagonal-crossing block
                    nc.vector.tensor_mul(
                        eT[:, 0, off : off + P], eT[:, 0, off : off + P], triu
                    )
                    nc.gpsimd.tensor_mul(
                        eT[:, 1, off : off + P], eT[:, 1, off : off + P], triu
                    )

                # emit the PREVIOUS step's AV matmuls now (one-step lag keeps
                # the in-order PE queue from stalling on this tile's exp)
                if pending is not None:
                    pending[0]()
                    if pending[1]:
                        pending[2]()

                # boundary filler: one out-projection tile every other step,
                # in the second half of the unit
                if op_tts and kt >= nkt - 2 * len(op_tts) and (nkt - kt) % 2 == 0:
                    emit_outproj_tt(op_tts.pop(0))

                def av(eT=eT, off=off, kt=kt, opss=opss, pair=pair, nkt=nkt):
                    for i in (0, 1):
                        nc.tensor.matmul(
                            opss[i][0:VW, off:QB],
                            lhsT=v_sb[:, kt, pair[i] * VW : (pair[i] + 1) * VW],
                            rhs=eT[:, i, off:QB],
                            start=(kt == 0),
                            stop=(kt == nkt - 1),
                        )

                def norm(hp=hp, qb=qb, opss=opss):
                    emit_norm(hp, qb, opss)

                pending = (av, kt == nkt - 1, norm)
        pending[0]()
        pending[2]()

        # last query block's out-projection
        for tt in range((NU - 1) * (QB // P), NU * (QB // P)):
            emit_outproj_tt(tt)

    nc.compile()
    return nc


def make_in_maps(x, W_qkv, b_qkv, W_out):
    x = np.asarray(x, np.float32)
    W_qkv = np.asarray(W_qkv, np.float32)
    b_qkv = np.asarray(b_qkv, np.float32)
    W_out = np.asarray(W_out, np.float32)
    in_maps = []
    for c in range(2 * B):
        b, g = divmod(c, 2)
        ch = g * HPC * HD
        wqa = W_qkv[:, ch : ch + 256].reshape(KT, P, 2 * P).transpose(1, 0, 2)
        wka = W_qkv[:, D + ch : D + ch + 256].reshape(KT, P, 2 * P).transpose(1, 0, 2)
        wqkb = np.concatenate(
            [
                b_qkv[ch : ch + 256].reshape(2, P).T,
                b_qkv[D + ch : D + ch + 256].reshape(2, P).T,
            ],
            axis=1,
        )  # [128, 4]: cols = q-m0, q-m1, k-m0, k-m1
        wva = np.zeros((D, HPC * VW), np.float32)
        wva3 = wva.reshape(D, HPC, VW)
        wva3[:, :, :HD] = W_qkv[:, 2 * D + ch : 2 * D + ch + 256].reshape(D, HPC, HD)
        wva = wva.reshape(KT, P, HPC * VW).transpose(1, 0, 2)
        wo = W_out[ch : ch + 256, :].reshape(2, P, D).transpose(1, 0, 2)
        in_maps.append(
            {
                "xT": np.ascontiguousarray(x[b].T).astype(_NP_MMDT),
                "wqa": np.ascontiguousarray(wqa).astype(_NP_MMDT),
                "wka": np.ascontiguousarray(wka).astype(_NP_MMDT),
                "wva": np.ascontiguousarray(wva).astype(_NP_MMDT),
                "wqkb": np.ascontiguousarray(wqkb, np.float32),
                "wo": np.ascontiguousarray(wo).astype(_NP_MMDT),
            }
        )
    return in_maps


def assemble(results, b_out, vbias_y):
    b_out = np.asarray(b_out, np.float32) + vbias_y
    out = np.empty((B, T, D), np.float32)
    for b in range(B):
        out[b] = (
            results[2 * b]["y"].astype(np.float32)
            + results[2 * b + 1]["y"].astype(np.float32)
            + b_out[None, :]
        )
    return out


_CACHE = {}


def kernel(x, W_qkv, b_qkv, W_out, b_out):
    if "nc" not in _CACHE:
        _CACHE["nc"] = build_bass()
    in_maps = make_in_maps(x, W_qkv, b_qkv, W_out)
    # v-bias contribution: softmax weights sum to 1, so b_v passes through
    # attention unchanged and lands as (b_v @ W_out) on every token.
    vbias_y = np.asarray(b_qkv, np.float32)[2 * D :] @ np.asarray(W_out, np.float32)
    res = run_bass_kernel_spmd(_CACHE["nc"], in_maps, list(range(2 * B)))
    return assemble(res.results, b_out, vbias_y)


# revision 15
# speedup vs baseline: 1.1833x; 1.1833x over previous
"""Multi-head causal self-attention on 8 Trainium2 NeuronCores.

Sharding: core c -> batch b = c // 2, heads 4*(c % 2) .. +4  (data parallel on
B, tensor parallel on heads).  Each core computes its 4 heads' attention for
its batch plus the partial out-projection; the host sums the two partials per
batch and adds b_out.

Per-core layout:
  xT   [D, T]      x[b] transposed on host (bf16)
  qT/kT [128, 2, T] head-major: partitions = 2 heads x 64, 2 m-tiles
  v    [128, 16, 260] natural [T, hd] per head + a ones column (gives the
                    softmax denominator for free during the AV matmul)
  scores are computed transposed: sT[k, q] = kT.T @ q, both heads of a pair
  packed into one [128, 2, 512] PSUM tile so a single evacuation call
  covers both; exp'd during PSUM->SBUF evacuation (no max subtraction:
  |scores| <= ~3 here), causality via tile skipping/trimming + one
  upper-triangular 128x128 mask constant.

Attention runs in 512-query units (qb, head-pair), query-block outer.  The
PE instruction stream is software-pipelined: AV matmuls lag the score
matmuls by one key-tile step (the PE queue is strictly in-order, so
without the lag every AV would stall on its own tile's exp evacuation),
AV accumulator PSUM sets alternate per unit, and each query block's
out-projection tiles are interleaved into the NEXT query block's score
stream as boundary filler.  The exp evacuation alternates by key-tile
parity between ACT (exact spline exp) and DVE (Schraudolph bit-trick exp2
via tensor_scalar -> int16 -> bf16 bitcast, ~3% per element, cancels in
softmax); triangular masks and normalization multiplies are split between
DVE and GpSimd.  The out-projection packs head pairs into K=128
accumulations (odd heads hop partitions via SBUF-to-SBUF DMA).
"""

import os
import sys
from contextlib import ExitStack

import numpy as np

for _p in ("/opt/trn_rl_repo", "/opt/pypackages"):
    if os.path.isdir(_p) and _p not in sys.path:
        sys.path.append(_p)

import concourse.bass as bass
from concourse import bacc
import concourse.mybir as mybir
import concourse.tile as tile
from concourse.bass_utils import run_bass_kernel_spmd
from concourse.masks import make_upper_triangular


B, T, D = 4, 2048, 512
H, HD = 8, 64
HPC = 4  # heads per core
P = 128
KT = D // P  # k-tiles over the model dim
QB = 512  # query-unit width / psum bank width
NKT = T // P  # key tiles
NU = T // QB  # query blocks
VW = HD + 1  # v columns per head incl. the ones column

F32 = mybir.dt.float32
I16 = mybir.dt.int16
BF16 = mybir.dt.bfloat16
MMDT = BF16
EXP = mybir.ActivationFunctionType.Exp

# fast-exp constants: exp(s/8) ~= bitcast_bf16(int16(s * FE_A + FE_B))
_LOG2E = 1.4426950408889634
FE_A = _LOG2E * 128.0 / 8.0
FE_B = 127.0 * 128.0 - 5.6

try:
    import ml_dtypes
    _NP_MMDT = np.dtype(ml_dtypes.bfloat16)
except ImportError:
    _NP_MMDT = np.float32


def build_bass():
    nc = bacc.Bacc()
    xT = nc.declare_dram_parameter("xT", [D, T], MMDT, isOutput=False)
    wqa = nc.declare_dram_parameter("wqa", [P, KT, 2 * P], MMDT, isOutput=False)
    wka = nc.declare_dram_parameter("wka", [P, KT, 2 * P], MMDT, isOutput=False)
    # q/k biases, laid out [channel % 128, channel // 128] for ACT bias APs
    wqkb = nc.declare_dram_parameter("wqkb", [P, 4], F32, isOutput=False)
    wva = nc.declare_dram_parameter("wva", [P, KT, HPC * VW], MMDT, isOutput=False)
    wo = nc.declare_dram_parameter("wo", [P, 2, D], MMDT, isOutput=False)
    y = nc.declare_dram_parameter("y", [T, D], BF16, isOutput=True)

    with tile.TileContext(nc) as tc, ExitStack() as ctx:
        consts = ctx.enter_context(tc.tile_pool(name="consts", bufs=1))
        qkv = ctx.enter_context(tc.tile_pool(name="qkv", bufs=1))
        attn = ctx.enter_context(tc.tile_pool(name="attn", bufs=1))
        etp = ctx.enter_context(tc.tile_pool(name="etp", bufs=4))
        nrm = ctx.enter_context(tc.tile_pool(name="nrm", bufs=4))
        yevac = ctx.enter_context(tc.tile_pool(name="yevac", bufs=3))
        # PSUM: "mm" 2 bufs x 4KB/partition (2 banks each) = 4 banks;
        # o{i}{s} 4 x [128,512]f32 (1 bank each) = 4 banks.  AV accumulator
        # sets s alternate per unit; QKV + out-proj borrow "mm".
        mmps = ctx.enter_context(tc.tile_pool(name="mmps", bufs=2, space="PSUM"))
        aps = ctx.enter_context(tc.tile_pool(name="aps", bufs=1, space="PSUM"))

        # ---- inputs / constants into SBUF (ordered for early compute start;
        # issue spread across the three DMA-capable engine queues)
        x_sb = consts.tile([P, KT, T], MMDT)
        wq_sb = consts.tile([P, KT, 2 * P], MMDT)
        wk_sb = consts.tile([P, KT, 2 * P], MMDT)
        wv_sb = consts.tile([P, KT, HPC * VW], MMDT)
        wqkb_sb = consts.tile([P, 4], F32)
        wo_sb = consts.tile([P, 2, D], MMDT)

        dmae = (nc.sync, nc.scalar, nc.gpsimd)
        nc.sync.dma_start(out=wq_sb, in_=wqa[:])
        nc.scalar.dma_start(out=wk_sb, in_=wka[:])
        n = 0
        for ch in range(4):  # 512-col chunks so the first matmul starts early
            for kt in range(KT):
                dmae[n % 3].dma_start(
                    out=x_sb[:, kt, ch * QB : (ch + 1) * QB],
                    in_=xT[kt * P : (kt + 1) * P, ch * QB : (ch + 1) * QB],
                )
                n += 1
            if ch == 0:
                nc.gpsimd.dma_start(out=wqkb_sb, in_=wqkb[:])
            if ch == 1:
                nc.scalar.dma_start(out=wv_sb, in_=wva[:])
        nc.gpsimd.dma_start(out=wo_sb, in_=wo[:])

        # triu[k, q] = 1 iff q >= k: allowed region of a diagonal block in
        # transposed-score space.  gpsimd affine_select needs f32; cast after.
        triu_st = consts.tile([P, P], F32)
        make_upper_triangular(nc, triu_st, val=1.0, diag=True)
        triu = consts.tile([P, P], MMDT)
        nc.vector.tensor_copy(triu, triu_st)

        # ---- QKV projections
        qT_sb = qkv.tile([P, 2, T], MMDT)
        kT_sb = qkv.tile([P, 2, T], MMDT)
        v_sb = qkv.tile([P, NKT, HPC * VW], MMDT)

        WB = 1024  # bf16 moving-operand max

        def qk_proj(wi, w_sb, dst, m, nb):
            ps = mmps.tile([P, WB], F32, tag="mm", name="ps")
            for lo in range(0, WB, QB):  # psum-bank-sized writes
                for kt in range(KT):
                    nc.tensor.matmul(
                        ps[:, lo : lo + QB],
                        lhsT=w_sb[:, kt, m * P : (m + 1) * P],
                        rhs=x_sb[:, kt, nb * WB + lo : nb * WB + lo + QB],
                        start=(kt == 0),
                        stop=(kt == KT - 1),
                    )
            nc.scalar.activation(
                out=dst[:, m, nb * WB : (nb + 1) * WB], in_=ps,
                func=mybir.ActivationFunctionType.Identity,
                bias=wqkb_sb[:, 2 * wi + m : 2 * wi + m + 1],
            )

        # v bias is folded into the host-side output bias (b_v @ W_out adds a
        # constant row after softmax-normalize + out-projection), so v here is
        # bias-free; the denominator ones-columns are memset directly.
        def v_proj(tt):
            tag = f"o{tt % 2}{'ab'[(tt // 2) % 2]}"
            ps = aps.tile([P, QB], F32, tag=tag, name="vps")
            for kt in range(KT):
                nc.tensor.matmul(
                    ps[:, 0 : HPC * VW],
                    lhsT=x_sb[:, kt, tt * P : (tt + 1) * P],
                    rhs=wv_sb[:, kt, :],
                    start=(kt == 0),
                    stop=(kt == KT - 1),
                )
            nc.scalar.copy(v_sb[:, tt, :], ps[:, 0 : HPC * VW])
            ones_cols = v_sb[:, tt, :].rearrange("p (h w) -> p h w", w=VW)[:, :, HD]
            nc.gpsimd.memset(ones_cols, 1.0)

        # issue order: everything the first attention units need first
        for m in range(2):
            qk_proj(0, wq_sb, qT_sb, m, 0)
            qk_proj(1, wk_sb, kT_sb, m, 0)
        for tt in range(NKT // 2):
            v_proj(tt)
        for m in range(2):
            qk_proj(0, wq_sb, qT_sb, m, 1)
            qk_proj(1, wk_sb, kT_sb, m, 1)
        for tt in range(NKT // 2, NKT):
            v_proj(tt)

        # ---- attention in 512-query units, PE-stream software-pipelined
        # attn pair tiles: rows 0-63 = even head, 64-127 = odd head of pair
        attn_p = [
            attn.tile([P, T], MMDT, tag=f"attnp{hp}", name=f"attnp{hp}")
            for hp in range(2)
        ]

        def emit_norm(hp, qb, opss):
            # normalize: row HD of ops is the softmax denominator.  The
            # copy evacuates the AV psum immediately (frees the bank); the
            # rest of the chain runs off SBUF.
            cols = slice(qb * QB, (qb + 1) * QB)
            for i in (0, 1):
                att_sb = nrm.tile([VW, QB], F32, tag="att", name="att")
                if i == 0:
                    nc.vector.tensor_copy(att_sb, opss[i][0:VW, :])
                else:
                    nc.scalar.copy(att_sb, opss[i][0:VW, :])
                rec = nrm.tile([VW, QB], F32, tag="rec", name="rec")
                nc.vector.reciprocal_approx_fast(out=rec, in_=att_sb)
                den0 = nrm.tile([1, QB], F32, tag="den0", name="den0")
                nc.sync.dma_start(out=den0, in_=rec[HD : HD + 1, :])
                bc = nrm.tile([HD, QB], F32, tag="bc", name="bc")
                nc.gpsimd.partition_broadcast(bc, den0)
                if i == 0:
                    nc.vector.tensor_mul(
                        attn_p[hp][0:HD, cols], att_sb[0:HD, :], bc
                    )
                else:
                    # odd head: normalize into a scratch at lanes 0-63,
                    # then DMA-hop to lanes 64-127 of the pair tile
                    odd = nrm.tile([HD, QB], MMDT, tag="odd", name="odd")
                    nc.vector.tensor_mul(odd, att_sb[0:HD, :], bc)
                    nc.sync.dma_start(out=attn_p[hp][HD:P, cols], in_=odd)

        def emit_outproj_tt(tt):
            # y[t, d] = attn_p0.T @ wo_p0 + attn_p1.T @ wo_p1  (K=128 each)
            ps = mmps.tile([P, D], F32, tag="mm", name="yps")
            for hp in range(2):
                nc.tensor.matmul(
                    ps,
                    lhsT=attn_p[hp][:, tt * P : (tt + 1) * P],
                    rhs=wo_sb[:, hp, :],
                    start=(hp == 0),
                    stop=(hp == 1),
                )
            yt = yevac.tile([P, D], BF16, tag="yt", name="yt")
            if tt % 2:
                nc.scalar.copy(yt, ps)
            else:
                nc.vector.tensor_copy(yt, ps)
            nc.sync.dma_start(out=y[tt * P : (tt + 1) * P, :], in_=yt)

        units = [(qb, hp) for qb in range(NU) for hp in range(HPC // 2)]
        pending = None  # (av_emitter, is_last_kt, norm_emitter)
        for u, (qb, hp) in enumerate(units):
            pair = (2 * hp, 2 * hp + 1)
            qhs = [
                qT_sb[(h % 2) * HD : (h % 2) * HD + HD, h // 2, :] for h in pair
            ]
            khs = [
                kT_sb[(h % 2) * HD : (h % 2) * HD + HD, h // 2, :] for h in pair
            ]
            st = "ab"[u % 2]
            opss = [
                aps.tile([P, QB], F32, tag=f"o{i}{st}", name=f"o{i}{st}")
                for i in range(2)
            ]
            nkt = (qb + 1) * (QB // P)
            # interleave the PREVIOUS query block's out-projection tiles into
            # this unit's score stream (hp==1 units, in the last steps --
            # late enough that the previous block's normalization chain,
            # including the odd-head partition-hop DMA, has long drained)
            op_tts = (
                list(range((qb - 1) * (QB // P), qb * (QB // P)))
                if hp == 1 and qb >= 1
                else []
            )
            for kt in range(nkt):
                off = max(0, kt * P - qb * QB)
                # scores for both heads into one [128, 2, 512] psum tile
                sps = mmps.tile([P, 2, QB], F32, tag="mm", name="sps")
                for i in (0, 1):
                    nc.tensor.matmul(
                        sps[:, i, off:QB],
                        lhsT=khs[i][:, kt * P : (kt + 1) * P],
                        rhs=qhs[i][:, qb * QB + off : (qb + 1) * QB],
                        start=True,
                        stop=True,
                    )
                eT = etp.tile([P, 2, QB], MMDT, tag="eT", name="eT")
                if kt % 2:  # DVE fast-exp on odd key tiles
                    nc.vector.tensor_scalar(
                        out=eT[:, :, off:QB].bitcast(I16),
                        in0=sps[:, :, off:QB],
                        scalar1=FE_A, scalar2=FE_B,
                        op0=mybir.AluOpType.mult,
                        op1=mybir.AluOpType.add,
                    )
                else:
                    nc.scalar.activation(
                        out=eT[:, :, off:QB], in_=sps[:, :, off:QB],
                        func=EXP, scale=1.0 / np.sqrt(HD),
                    )
                if kt * P >= qb * QB:  # diagonal-crossing block
                    nc.vector.tensor_mul(
                        eT[:, 0, off : off + P], eT[:, 0, off : off + P], triu
                    )
                    nc.gpsimd.tensor_mul(
                        eT[:, 1, off : off + P], eT[:, 1, off : off + P], triu
                    )

                # emit the PREVIOUS step's AV matmuls now (one-step lag keeps
                # the in-order PE queue from stalling on this tile's exp)
                if pending is not None:
                    pending[0]()
                    if pending[1]:
                        pending[2]()

                # boundary filler: one out-projection tile per step in the
                # unit's last steps
                if op_tts and kt >= nkt - len(op_tts):
                    emit_outproj_tt(op_tts.pop(0))

                def av(eT=eT, off=off, kt=kt, opss=opss, pair=pair, nkt=nkt):
                    for i in (0, 1):
                        nc.tensor.matmul(
                            opss[i][0:VW, off:QB],
                            lhsT=v_sb[:, kt, pair[i] * VW : (pair[i] + 1) * VW],
                            rhs=eT[:, i, off:QB],
                            start=(kt == 0),
                            stop=(kt == nkt - 1),
                        )

                def norm(hp=hp, qb=qb, opss=opss):
                    emit_norm(hp, qb, opss)

                pending = (av, kt == nkt - 1, norm)
        pending[0]()
        pending[2]()

        # last query block's out-projection
        for tt in range((NU - 1) * (QB // P), NU * (QB // P)):
            emit_outproj_tt(tt)

    nc.compile()
    return nc


def make_in_maps(x, W_qkv, b_qkv, W_out):
    x = np.asarray(x, np.float32)
    W_qkv = np.asarray(W_qkv, np.float32)
    b_qkv = np.asarray(b_qkv, np.float32)
    W_out = np.asarray(W_out, np.float32)
    in_maps = []
    for c in range(2 * B):
        b, g = divmod(c, 2)
        ch = g * HPC * HD
        wqa = W_qkv[:, ch : ch + 256].reshape(KT, P, 2 * P).transpose(1, 0, 2)
        wka = W_qkv[:, D + ch : D + ch + 256].reshape(KT, P, 2 * P).transpose(1, 0, 2)
        wqkb = np.concatenate(
            [
                b_qkv[ch : ch + 256].reshape(2, P).T,
                b_qkv[D + ch : D + ch + 256].reshape(2, P).T,
            ],
            axis=1,
        )  # [128, 4]: cols = q-m0, q-m1, k-m0, k-m1
        wva = np.zeros((D, HPC * VW), np.float32)
        wva3 = wva.reshape(D, HPC, VW)
        wva3[:, :, :HD] = W_qkv[:, 2 * D + ch : 2 * D + ch + 256].reshape(D, HPC, HD)
        wva = wva.reshape(KT, P, HPC * VW).transpose(1, 0, 2)
        wo = W_out[ch : ch + 256, :].reshape(2, P, D).transpose(1, 0, 2)
        in_maps.append(
            {
                "xT": np.ascontiguousarray(x[b].T).astype(_NP_MMDT),
                "wqa": np.ascontiguousarray(wqa).astype(_NP_MMDT),
                "wka": np.ascontiguousarray(wka).astype(_NP_MMDT),
                "wva": np.ascontiguousarray(wva).astype(_NP_MMDT),
                "wqkb": np.ascontiguousarray(wqkb, np.float32),
                "wo": np.ascontiguousarray(wo).astype(_NP_MMDT),
            }
        )
    return in_maps


def assemble(results, b_out, vbias_y):
    b_out = np.asarray(b_out, np.float32) + vbias_y
    out = np.empty((B, T, D), np.float32)
    for b in range(B):
        out[b] = (
            results[2 * b]["y"].astype(np.float32)
            + results[2 * b + 1]["y"].astype(np.float32)
            + b_out[None, :]
        )
    return out


_CACHE = {}


def kernel(x, W_qkv, b_qkv, W_out, b_out):
    if "nc" not in _CACHE:
        _CACHE["nc"] = build_bass()
    in_maps = make_in_maps(x, W_qkv, b_qkv, W_out)
    # v-bias contribution: softmax weights sum to 1, so b_v passes through
    # attention unchanged and lands as (b_v @ W_out) on every token.
    vbias_y = np.asarray(b_qkv, np.float32)[2 * D :] @ np.asarray(W_out, np.float32)
    res = run_bass_kernel_spmd(_CACHE["nc"], in_maps, list(range(2 * B)))
    return assemble(res.results, b_out, vbias_y)


# revision 16
# speedup vs baseline: 1.2526x; 1.0585x over previous
"""Multi-head causal self-attention on 8 Trainium2 NeuronCores.

Sharding: core c -> batch b = c // 2, heads 4*(c % 2) .. +4  (data parallel on
B, tensor parallel on heads).  Each core computes its 4 heads' attention for
its batch plus the partial out-projection; the host sums the two partials per
batch and adds b_out.

Per-core layout:
  xT   [D, T]      x[b] transposed on host (bf16)
  qT/kT [128, 2, T] head-major: partitions = 2 heads x 64, 2 m-tiles
  v    [128, 16, 260] natural [T, hd] per head + a ones column (gives the
                    softmax denominator for free during the AV matmul)
  scores are computed transposed: sT[k, q] = kT.T @ q, both heads of a pair
  packed into one [128, 2, 512] PSUM tile so a single evacuation call
  covers both; exp'd during PSUM->SBUF evacuation (no max subtraction:
  |scores| <= ~3 here), causality via tile skipping/trimming + one
  upper-triangular 128x128 mask constant.

Attention runs in 512-query units (qb, head-pair), query-block outer.  The
PE instruction stream is software-pipelined: AV matmuls lag the score
matmuls by one key-tile step (the PE queue is strictly in-order, so
without the lag every AV would stall on its own tile's exp evacuation),
AV accumulator PSUM sets alternate per unit, and each query block's
out-projection tiles are interleaved into the NEXT query block's score
stream as boundary filler.  The exp evacuation alternates by key-tile
parity between ACT (exact spline exp) and DVE (Schraudolph bit-trick exp2
via tensor_scalar -> int16 -> bf16 bitcast, ~3% per element, cancels in
softmax); triangular masks and normalization multiplies are split between
DVE and GpSimd.  The out-projection packs head pairs into K=128
accumulations (odd heads hop partitions via SBUF-to-SBUF DMA).
"""

import os
import sys
from contextlib import ExitStack

import numpy as np

for _p in ("/opt/trn_rl_repo", "/opt/pypackages"):
    if os.path.isdir(_p) and _p not in sys.path:
        sys.path.append(_p)

import concourse.bass as bass
from concourse import bacc
import concourse.mybir as mybir
import concourse.tile as tile
from concourse.bass_utils import run_bass_kernel_spmd
from concourse.masks import make_upper_triangular


B, T, D = 4, 2048, 512
H, HD = 8, 64
HPC = 4  # heads per core
P = 128
KT = D // P  # k-tiles over the model dim
QB = 512  # query-unit width / psum bank width
NKT = T // P  # key tiles
NU = T // QB  # query blocks
VW = HD + 1  # v columns per head incl. the ones column

F32 = mybir.dt.float32
I16 = mybir.dt.int16
BF16 = mybir.dt.bfloat16
MMDT = BF16
EXP = mybir.ActivationFunctionType.Exp

# fast-exp constants: exp(s/8) ~= bitcast_bf16(int16(s * FE_A + FE_B))
_LOG2E = 1.4426950408889634
FE_A = _LOG2E * 128.0 / 8.0
FE_B = 127.0 * 128.0 - 5.6

try:
    import ml_dtypes
    _NP_MMDT = np.dtype(ml_dtypes.bfloat16)
except ImportError:
    _NP_MMDT = np.float32


def build_bass():
    nc = bacc.Bacc()
    xT = nc.declare_dram_parameter("xT", [D, T], MMDT, isOutput=False)
    wqa = nc.declare_dram_parameter("wqa", [P, KT, 2 * P], MMDT, isOutput=False)
    wka = nc.declare_dram_parameter("wka", [P, KT, 2 * P], MMDT, isOutput=False)
    # q/k biases, laid out [channel % 128, channel // 128] for ACT bias APs
    wqkb = nc.declare_dram_parameter("wqkb", [P, 4], F32, isOutput=False)
    wva = nc.declare_dram_parameter("wva", [P, KT, HPC * VW], MMDT, isOutput=False)
    wo = nc.declare_dram_parameter("wo", [P, 2, D], MMDT, isOutput=False)
    y = nc.declare_dram_parameter("y", [T, D], BF16, isOutput=True)

    with tile.TileContext(nc) as tc, ExitStack() as ctx:
        consts = ctx.enter_context(tc.tile_pool(name="consts", bufs=1))
        qkv = ctx.enter_context(tc.tile_pool(name="qkv", bufs=1))
        attn = ctx.enter_context(tc.tile_pool(name="attn", bufs=1))
        etp = ctx.enter_context(tc.tile_pool(name="etp", bufs=4))
        nrm = ctx.enter_context(tc.tile_pool(name="nrm", bufs=4))
        yevac = ctx.enter_context(tc.tile_pool(name="yevac", bufs=3))
        # PSUM: "mm" 2 bufs x 4KB/partition (2 banks each) = 4 banks;
        # o{i}{s} 4 x [128,512]f32 (1 bank each) = 4 banks.  AV accumulator
        # sets s alternate per unit; QKV + out-proj borrow "mm".
        mmps = ctx.enter_context(tc.tile_pool(name="mmps", bufs=2, space="PSUM"))
        aps = ctx.enter_context(tc.tile_pool(name="aps", bufs=1, space="PSUM"))

        # ---- inputs / constants into SBUF (ordered for early compute start;
        # issue spread across the three DMA-capable engine queues)
        x_sb = consts.tile([P, KT, T], MMDT)
        wq_sb = consts.tile([P, KT, 2 * P], MMDT)
        wk_sb = consts.tile([P, KT, 2 * P], MMDT)
        wv_sb = consts.tile([P, KT, HPC * VW], MMDT)
        wqkb_sb = consts.tile([P, 4], F32)
        wo_sb = consts.tile([P, 2, D], MMDT)

        dmae = (nc.sync, nc.scalar, nc.gpsimd)
        nc.sync.dma_start(out=wq_sb, in_=wqa[:])
        nc.scalar.dma_start(out=wk_sb, in_=wka[:])
        n = 0
        for ch in range(4):  # 512-col chunks so the first matmul starts early
            for kt in range(KT):
                dmae[n % 3].dma_start(
                    out=x_sb[:, kt, ch * QB : (ch + 1) * QB],
                    in_=xT[kt * P : (kt + 1) * P, ch * QB : (ch + 1) * QB],
                )
                n += 1
            if ch == 0:
                nc.gpsimd.dma_start(out=wqkb_sb, in_=wqkb[:])
            if ch == 1:
                nc.scalar.dma_start(out=wv_sb, in_=wva[:])
        nc.gpsimd.dma_start(out=wo_sb, in_=wo[:])

        # triu[k, q] = 1 iff q >= k: allowed region of a diagonal block in
        # transposed-score space.  gpsimd affine_select needs f32; cast after.
        triu_st = consts.tile([P, P], F32)
        make_upper_triangular(nc, triu_st, val=1.0, diag=True)
        triu = consts.tile([P, P], MMDT)
        nc.vector.tensor_copy(triu, triu_st)

        # ---- QKV projections
        qT_sb = qkv.tile([P, 2, T], MMDT)
        kT_sb = qkv.tile([P, 2, T], MMDT)
        v_sb = qkv.tile([P, NKT, HPC * VW], MMDT)

        WB = 1024  # bf16 moving-operand max

        def qk_proj(wi, w_sb, dst, m, nb):
            ps = mmps.tile([P, WB], F32, tag="mm", name="ps")
            for lo in range(0, WB, QB):  # psum-bank-sized writes
                for kt in range(KT):
                    nc.tensor.matmul(
                        ps[:, lo : lo + QB],
                        lhsT=w_sb[:, kt, m * P : (m + 1) * P],
                        rhs=x_sb[:, kt, nb * WB + lo : nb * WB + lo + QB],
                        start=(kt == 0),
                        stop=(kt == KT - 1),
                    )
            nc.scalar.activation(
                out=dst[:, m, nb * WB : (nb + 1) * WB], in_=ps,
                func=mybir.ActivationFunctionType.Identity,
                bias=wqkb_sb[:, 2 * wi + m : 2 * wi + m + 1],
            )

        # v bias is folded into the host-side output bias (b_v @ W_out adds a
        # constant row after softmax-normalize + out-projection), so v here is
        # bias-free; the denominator ones-columns are memset directly.
        def v_proj(tt):
            tag = f"o{tt % 2}{'ab'[(tt // 2) % 2]}"
            ps = aps.tile([P, QB], F32, tag=tag, name="vps")
            for kt in range(KT):
                nc.tensor.matmul(
                    ps[:, 0 : HPC * VW],
                    lhsT=x_sb[:, kt, tt * P : (tt + 1) * P],
                    rhs=wv_sb[:, kt, :],
                    start=(kt == 0),
                    stop=(kt == KT - 1),
                )
            nc.scalar.copy(v_sb[:, tt, :], ps[:, 0 : HPC * VW])
            ones_cols = v_sb[:, tt, :].rearrange("p (h w) -> p h w", w=VW)[:, :, HD]
            nc.gpsimd.memset(ones_cols, 1.0)

        # issue order: everything the first attention units need first
        for m in range(2):
            qk_proj(0, wq_sb, qT_sb, m, 0)
            qk_proj(1, wk_sb, kT_sb, m, 0)
        for tt in range(NKT // 2):
            v_proj(tt)
        for m in range(2):
            qk_proj(0, wq_sb, qT_sb, m, 1)
            qk_proj(1, wk_sb, kT_sb, m, 1)
        for tt in range(NKT // 2, NKT):
            v_proj(tt)

        # ---- attention in 512-query units, PE-stream software-pipelined
        # attn pair tiles: rows 0-63 = even head, 64-127 = odd head of pair
        attn_p = [
            attn.tile([P, T], MMDT, tag=f"attnp{hp}", name=f"attnp{hp}")
            for hp in range(2)
        ]

        # Deferred-emission scheduler: every latency-chained op (the
        # normalization stages, out-projection) is emitted several PE steps
        # after its inputs were produced, so no in-order engine queue ever
        # blocks on an unmet dependency in front of PE-feeding work.
        import heapq
        import itertools

        step = 0
        seq = itertools.count()
        deferred = []  # heap of (due_step, seq, fn)

        def defer(delta, fn):
            heapq.heappush(deferred, (step + delta, next(seq), fn))

        def flush():
            while deferred and deferred[0][0] <= step:
                heapq.heappop(deferred)[2]()

        def emit_outproj_tt(tt):
            # y[t, d] = attn_p0.T @ wo_p0 + attn_p1.T @ wo_p1  (K=128 each)
            ps = mmps.tile([P, D], F32, tag="mm", name="yps")
            for hp in range(2):
                nc.tensor.matmul(
                    ps,
                    lhsT=attn_p[hp][:, tt * P : (tt + 1) * P],
                    rhs=wo_sb[:, hp, :],
                    start=(hp == 0),
                    stop=(hp == 1),
                )
            yt = yevac.tile([P, D], BF16, tag="yt", name="yt")
            if tt % 2:
                nc.scalar.copy(yt, ps)
            else:
                nc.vector.tensor_copy(yt, ps)
            nc.sync.dma_start(out=y[tt * P : (tt + 1) * P, :], in_=yt)

        def schedule_norm(hp, qb, opss):
            # normalization pipeline: row HD of ops is the softmax
            # denominator.  Stage emissions are spread over subsequent PE
            # steps (~0.6us each) so every op's inputs are ready before its
            # engine reaches it.
            cols = slice(qb * QB, (qb + 1) * QB)
            att0 = nrm.tile([VW, QB], F32, tag="att0", name="att0")
            att1 = nrm.tile([VW, QB], F32, tag="att1", name="att1")
            rec0 = nrm.tile([VW, QB], F32, tag="rec0", name="rec0")
            rec1 = nrm.tile([VW, QB], F32, tag="rec1", name="rec1")
            den0 = nrm.tile([1, QB], F32, tag="den0", name="den0")
            den1 = nrm.tile([1, QB], F32, tag="den1", name="den1")
            bc0 = nrm.tile([HD, QB], F32, tag="bc0", name="bc0")
            bc1 = nrm.tile([HD, QB], F32, tag="bc1", name="bc1")
            odd = nrm.tile([HD, QB], MMDT, tag="odd", name="odd")

            def s_copy():
                # evacuate the AV psum (frees the banks for the next-next
                # unit); DVE takes the even head, ACT the odd
                nc.vector.tensor_copy(att0, opss[0][0:VW, :])
                nc.scalar.copy(att1, opss[1][0:VW, :])

            def s_recip():
                nc.vector.reciprocal_approx_fast(out=rec0, in_=att0)
                nc.vector.reciprocal_approx_fast(out=rec1, in_=att1)
                nc.sync.dma_start(out=den0, in_=rec0[HD : HD + 1, :])
                nc.sync.dma_start(out=den1, in_=rec1[HD : HD + 1, :])

            def s_bcast():
                nc.gpsimd.partition_broadcast(bc0, den0)
                nc.gpsimd.partition_broadcast(bc1, den1)

            def s_mul():
                nc.vector.tensor_mul(attn_p[hp][0:HD, cols], att0[0:HD, :], bc0)
                # odd head: normalize into a scratch at lanes 0-63 (GpSimd;
                # SBUF-only operands), then DMA-hop to lanes 64-127
                nc.gpsimd.tensor_mul(odd, att1[0:HD, :], bc1)
                nc.sync.dma_start(out=attn_p[hp][HD:P, cols], in_=odd)

            defer(2, s_copy)
            defer(4, s_recip)
            defer(6, s_bcast)
            defer(8, s_mul)
            if hp == 1:
                for k, tt in enumerate(range(qb * (QB // P), (qb + 1) * (QB // P))):
                    defer(12 + k, lambda tt=tt: emit_outproj_tt(tt))

        units = [(qb, hp) for qb in range(NU) for hp in range(HPC // 2)]
        pending = None  # av_emitter from the previous step
        for u, (qb, hp) in enumerate(units):
            pair = (2 * hp, 2 * hp + 1)
            qhs = [
                qT_sb[(h % 2) * HD : (h % 2) * HD + HD, h // 2, :] for h in pair
            ]
            khs = [
                kT_sb[(h % 2) * HD : (h % 2) * HD + HD, h // 2, :] for h in pair
            ]
            st = "ab"[u % 2]
            opss = [
                aps.tile([P, QB], F32, tag=f"o{i}{st}", name=f"o{i}{st}")
                for i in range(2)
            ]
            nkt = (qb + 1) * (QB // P)
            for kt in range(nkt):
                off = max(0, kt * P - qb * QB)
                # scores for both heads into one [128, 2, 512] psum tile
                sps = mmps.tile([P, 2, QB], F32, tag="mm", name="sps")
                for i in (0, 1):
                    nc.tensor.matmul(
                        sps[:, i, off:QB],
                        lhsT=khs[i][:, kt * P : (kt + 1) * P],
                        rhs=qhs[i][:, qb * QB + off : (qb + 1) * QB],
                        start=True,
                        stop=True,
                    )
                eT = etp.tile([P, 2, QB], MMDT, tag="eT", name="eT")
                if kt % 2:  # DVE fast-exp on odd key tiles
                    nc.vector.tensor_scalar(
                        out=eT[:, :, off:QB].bitcast(I16),
                        in0=sps[:, :, off:QB],
                        scalar1=FE_A, scalar2=FE_B,
                        op0=mybir.AluOpType.mult,
                        op1=mybir.AluOpType.add,
                    )
                else:
                    nc.scalar.activation(
                        out=eT[:, :, off:QB], in_=sps[:, :, off:QB],
                        func=EXP, scale=1.0 / np.sqrt(HD),
                    )
                if kt * P >= qb * QB:  # diagonal-crossing block (DVE: the
                    # masks feed the PE next step, keep them off queues with
                    # waiting ops)
                    for i in (0, 1):
                        nc.vector.tensor_mul(
                            eT[:, i, off : off + P], eT[:, i, off : off + P], triu
                        )

                # emit the PREVIOUS step's AV matmuls now (one-step lag keeps
                # the in-order PE queue from stalling on this tile's exp),
                # then any due deferred work
                if pending is not None:
                    pending()
                flush()

                def av(eT=eT, off=off, kt=kt, opss=opss, pair=pair, nkt=nkt,
                       hp=hp, qb=qb):
                    for i in (0, 1):
                        nc.tensor.matmul(
                            opss[i][0:VW, off:QB],
                            lhsT=v_sb[:, kt, pair[i] * VW : (pair[i] + 1) * VW],
                            rhs=eT[:, i, off:QB],
                            start=(kt == 0),
                            stop=(kt == nkt - 1),
                        )
                    if kt == nkt - 1:
                        schedule_norm(hp, qb, opss)

                pending = av
                step += 1
        pending()
        while deferred:
            heapq.heappop(deferred)[2]()

    nc.compile()
    return nc


def make_in_maps(x, W_qkv, b_qkv, W_out):
    x = np.asarray(x, np.float32)
    W_qkv = np.asarray(W_qkv, np.float32)
    b_qkv = np.asarray(b_qkv, np.float32)
    W_out = np.asarray(W_out, np.float32)
    in_maps = []
    for c in range(2 * B):
        b, g = divmod(c, 2)
        ch = g * HPC * HD
        wqa = W_qkv[:, ch : ch + 256].reshape(KT, P, 2 * P).transpose(1, 0, 2)
        wka = W_qkv[:, D + ch : D + ch + 256].reshape(KT, P, 2 * P).transpose(1, 0, 2)
        wqkb = np.concatenate(
            [
                b_qkv[ch : ch + 256].reshape(2, P).T,
                b_qkv[D + ch : D + ch + 256].reshape(2, P).T,
            ],
            axis=1,
        )  # [128, 4]: cols = q-m0, q-m1, k-m0, k-m1
        wva = np.zeros((D, HPC * VW), np.float32)
        wva3 = wva.reshape(D, HPC, VW)
        wva3[:, :, :HD] = W_qkv[:, 2 * D + ch : 2 * D + ch + 256].reshape(D, HPC, HD)
        wva = wva.reshape(KT, P, HPC * VW).transpose(1, 0, 2)
        wo = W_out[ch : ch + 256, :].reshape(2, P, D).transpose(1, 0, 2)
        in_maps.append(
            {
                "xT": np.ascontiguousarray(x[b].T).astype(_NP_MMDT),
                "wqa": np.ascontiguousarray(wqa).astype(_NP_MMDT),
                "wka": np.ascontiguousarray(wka).astype(_NP_MMDT),
                "wva": np.ascontiguousarray(wva).astype(_NP_MMDT),
                "wqkb": np.ascontiguousarray(wqkb, np.float32),
                "wo": np.ascontiguousarray(wo).astype(_NP_MMDT),
            }
        )
    return in_maps


def assemble(results, b_out, vbias_y):
    b_out = np.asarray(b_out, np.float32) + vbias_y
    out = np.empty((B, T, D), np.float32)
    for b in range(B):
        out[b] = (
            results[2 * b]["y"].astype(np.float32)
            + results[2 * b + 1]["y"].astype(np.float32)
            + b_out[None, :]
        )
    return out


_CACHE = {}


def kernel(x, W_qkv, b_qkv, W_out, b_out):
    if "nc" not in _CACHE:
        _CACHE["nc"] = build_bass()
    in_maps = make_in_maps(x, W_qkv, b_qkv, W_out)
    # v-bias contribution: softmax weights sum to 1, so b_v passes through
    # attention unchanged and lands as (b_v @ W_out) on every token.
    vbias_y = np.asarray(b_qkv, np.float32)[2 * D :] @ np.asarray(W_out, np.float32)
    res = run_bass_kernel_spmd(_CACHE["nc"], in_maps, list(range(2 * B)))
    return assemble(res.results, b_out, vbias_y)


# revision 19
# speedup vs baseline: 1.3086x; 1.0447x over previous
"""Multi-head causal self-attention on 8 Trainium2 NeuronCores.

Sharding: core c -> batch b = c // 2, heads 4*(c % 2) .. +4  (data parallel on
B, tensor parallel on heads).  Each core computes its 4 heads' attention for
its batch plus the partial out-projection; the host sums the two partials per
batch and adds b_out.

Per-core layout:
  xT   [D, T]      x[b] transposed on host (bf16)
  qT/kT [128, 2, T] head-major: partitions = 2 heads x 64, 2 m-tiles
  v    [128, 16, 260] natural [T, hd] per head + a ones column (gives the
                    softmax denominator for free during the AV matmul)
  scores are computed transposed: sT[k, q] = kT.T @ q, both heads of a pair
  packed into one [128, 2, 512] PSUM tile so a single evacuation call
  covers both; exp'd during PSUM->SBUF evacuation (no max subtraction:
  |scores| <= ~3 here), causality via tile skipping/trimming + one
  upper-triangular 128x128 mask constant.

Attention runs in 512-query units (qb, head-pair), query-block outer.  The
PE instruction stream is software-pipelined: AV matmuls lag the score
matmuls by one key-tile step (the PE queue is strictly in-order, so
without the lag every AV would stall on its own tile's exp evacuation),
AV accumulator PSUM sets alternate per unit, and each query block's
out-projection tiles are interleaved into the NEXT query block's score
stream as boundary filler.  The exp evacuation alternates by key-tile
parity between ACT (exact spline exp) and DVE (Schraudolph bit-trick exp2
via tensor_scalar -> int16 -> bf16 bitcast, ~3% per element, cancels in
softmax); triangular masks and normalization multiplies are split between
DVE and GpSimd.  The out-projection packs head pairs into K=128
accumulations (odd heads hop partitions via SBUF-to-SBUF DMA).
"""

import os
import sys
from contextlib import ExitStack

import numpy as np

for _p in ("/opt/trn_rl_repo", "/opt/pypackages"):
    if os.path.isdir(_p) and _p not in sys.path:
        sys.path.append(_p)

import concourse.bass as bass
from concourse import bacc
import concourse.mybir as mybir
import concourse.tile as tile
from concourse.bass_utils import run_bass_kernel_spmd
from concourse.masks import make_upper_triangular


B, T, D = 4, 2048, 512
H, HD = 8, 64
HPC = 4  # heads per core
P = 128
KT = D // P  # k-tiles over the model dim
QB = 512  # query-unit width / psum bank width
NKT = T // P  # key tiles
NU = T // QB  # query blocks
VW = HD + 1  # v columns per head incl. the ones column

F32 = mybir.dt.float32
I16 = mybir.dt.int16
BF16 = mybir.dt.bfloat16
MMDT = BF16
EXP = mybir.ActivationFunctionType.Exp

# fast-exp constants: exp(s/8) ~= bitcast_bf16(int16(s * FE_A + FE_B))
_LOG2E = 1.4426950408889634
FE_A = _LOG2E * 128.0 / 8.0
FE_B = 127.0 * 128.0 - 5.6

try:
    import ml_dtypes
    _NP_MMDT = np.dtype(ml_dtypes.bfloat16)
except ImportError:
    _NP_MMDT = np.float32


def build_bass():
    nc = bacc.Bacc()
    xT = nc.declare_dram_parameter("xT", [D, T], MMDT, isOutput=False)
    wqa = nc.declare_dram_parameter("wqa", [P, KT, 2 * P], MMDT, isOutput=False)
    wka = nc.declare_dram_parameter("wka", [P, KT, 2 * P], MMDT, isOutput=False)
    # q/k biases, laid out [channel % 128, channel // 128] for ACT bias APs
    wqkb = nc.declare_dram_parameter("wqkb", [P, 4], F32, isOutput=False)
    wva = nc.declare_dram_parameter("wva", [P, KT, HPC * VW], MMDT, isOutput=False)
    wo = nc.declare_dram_parameter("wo", [P, 2, D], MMDT, isOutput=False)
    y = nc.declare_dram_parameter("y", [T, D], BF16, isOutput=True)

    with tile.TileContext(nc) as tc, ExitStack() as ctx:
        consts = ctx.enter_context(tc.tile_pool(name="consts", bufs=1))
        qkv = ctx.enter_context(tc.tile_pool(name="qkv", bufs=1))
        attn = ctx.enter_context(tc.tile_pool(name="attn", bufs=1))
        etp = ctx.enter_context(tc.tile_pool(name="etp", bufs=4))
        nrm = ctx.enter_context(tc.tile_pool(name="nrm", bufs=4))
        yevac = ctx.enter_context(tc.tile_pool(name="yevac", bufs=3))
        # PSUM: "mm" 2 bufs x 4KB/partition (2 banks each) = 4 banks;
        # o{i}{s} 4 x [128,512]f32 (1 bank each) = 4 banks.  AV accumulator
        # sets s alternate per unit; QKV + out-proj borrow "mm".
        mmps = ctx.enter_context(tc.tile_pool(name="mmps", bufs=2, space="PSUM"))
        aps = ctx.enter_context(tc.tile_pool(name="aps", bufs=1, space="PSUM"))

        # ---- inputs / constants into SBUF (ordered for early compute start;
        # issue spread across the three DMA-capable engine queues)
        x_sb = consts.tile([P, KT, T], MMDT)
        wq_sb = consts.tile([P, KT, 2 * P], MMDT)
        wk_sb = consts.tile([P, KT, 2 * P], MMDT)
        wv_sb = consts.tile([P, KT, HPC * VW], MMDT)
        wqkb_sb = consts.tile([P, 4], F32)
        wo_sb = consts.tile([P, 2, D], MMDT)

        dmae = (nc.sync, nc.scalar, nc.gpsimd)
        nc.sync.dma_start(out=wq_sb, in_=wqa[:])
        nc.scalar.dma_start(out=wk_sb, in_=wka[:])
        n = 0
        for ch in range(4):  # 512-col chunks so the first matmul starts early
            for kt in range(KT):
                dmae[n % 3].dma_start(
                    out=x_sb[:, kt, ch * QB : (ch + 1) * QB],
                    in_=xT[kt * P : (kt + 1) * P, ch * QB : (ch + 1) * QB],
                )
                n += 1
            if ch == 0:
                nc.gpsimd.dma_start(out=wqkb_sb, in_=wqkb[:])
            if ch == 1:
                nc.scalar.dma_start(out=wv_sb, in_=wva[:])
        nc.gpsimd.dma_start(out=wo_sb, in_=wo[:])

        # triu[k, q] = 1 iff q >= k: allowed region of a diagonal block in
        # transposed-score space.  gpsimd affine_select needs f32; cast after.
        triu_st = consts.tile([P, P], F32)
        make_upper_triangular(nc, triu_st, val=1.0, diag=True)
        triu = consts.tile([P, P], MMDT)
        nc.vector.tensor_copy(triu, triu_st)

        # ---- QKV projections
        qT_sb = qkv.tile([P, 2, T], MMDT)
        kT_sb = qkv.tile([P, 2, T], MMDT)
        v_sb = qkv.tile([P, NKT, HPC * VW], MMDT)

        WB = 1024  # bf16 moving-operand max

        def qk_proj(wi, w_sb, dst, m, nb):
            ps = mmps.tile([P, WB], F32, tag="mm", name="ps")
            for lo in range(0, WB, QB):  # psum-bank-sized writes
                for kt in range(KT):
                    nc.tensor.matmul(
                        ps[:, lo : lo + QB],
                        lhsT=w_sb[:, kt, m * P : (m + 1) * P],
                        rhs=x_sb[:, kt, nb * WB + lo : nb * WB + lo + QB],
                        start=(kt == 0),
                        stop=(kt == KT - 1),
                    )
            nc.scalar.activation(
                out=dst[:, m, nb * WB : (nb + 1) * WB], in_=ps,
                func=mybir.ActivationFunctionType.Identity,
                bias=wqkb_sb[:, 2 * wi + m : 2 * wi + m + 1],
            )

        # v bias is folded into the host-side output bias (b_v @ W_out adds a
        # constant row after softmax-normalize + out-projection), so v here is
        # bias-free; the denominator ones-columns are memset directly.
        def v_proj(tt):
            tag = f"o{tt % 2}{'ab'[(tt // 2) % 2]}"
            ps = aps.tile([P, QB], F32, tag=tag, name="vps")
            for kt in range(KT):
                nc.tensor.matmul(
                    ps[:, 0 : HPC * VW],
                    lhsT=x_sb[:, kt, tt * P : (tt + 1) * P],
                    rhs=wv_sb[:, kt, :],
                    start=(kt == 0),
                    stop=(kt == KT - 1),
                )
            nc.scalar.copy(v_sb[:, tt, :], ps[:, 0 : HPC * VW])
            ones_cols = v_sb[:, tt, :].rearrange("p (h w) -> p h w", w=VW)[:, :, HD]
            nc.gpsimd.memset(ones_cols, 1.0)

        # issue order: everything the first attention units need first
        for m in range(2):
            qk_proj(0, wq_sb, qT_sb, m, 0)
            qk_proj(1, wk_sb, kT_sb, m, 0)
        for tt in range(NKT // 2):
            v_proj(tt)
        for m in range(2):
            qk_proj(0, wq_sb, qT_sb, m, 1)
            qk_proj(1, wk_sb, kT_sb, m, 1)
        for tt in range(NKT // 2, NKT):
            v_proj(tt)

        # ---- attention in 512-query units, PE-stream software-pipelined
        # attn pair tiles: rows 0-63 = even head, 64-127 = odd head of pair
        attn_p = [
            attn.tile([P, T], MMDT, tag=f"attnp{hp}", name=f"attnp{hp}")
            for hp in range(2)
        ]

        # Deferred-emission scheduler: every latency-chained op (the
        # normalization stages, out-projection) is emitted several PE steps
        # after its inputs were produced, so no in-order engine queue ever
        # blocks on an unmet dependency in front of PE-feeding work.
        import heapq
        import itertools

        step = 0
        seq = itertools.count()
        deferred = []  # heap of (due_step, seq, fn)

        def defer(delta, fn):
            heapq.heappush(deferred, (step + delta, next(seq), fn))

        def flush():
            while deferred and deferred[0][0] <= step:
                heapq.heappop(deferred)[2]()

        def emit_outproj_tt(tt):
            # y[t, d] = attn_p0.T @ wo_p0 + attn_p1.T @ wo_p1  (K=128 each)
            ps = mmps.tile([P, D], F32, tag="mm", name="yps")
            for hp in range(2):
                nc.tensor.matmul(
                    ps,
                    lhsT=attn_p[hp][:, tt * P : (tt + 1) * P],
                    rhs=wo_sb[:, hp, :],
                    start=(hp == 0),
                    stop=(hp == 1),
                )
            yt = yevac.tile([P, D], BF16, tag="yt", name="yt")
            if tt % 2:
                nc.scalar.copy(yt, ps)
            else:
                nc.vector.tensor_copy(yt, ps)
            nc.sync.dma_start(out=y[tt * P : (tt + 1) * P, :], in_=yt)

        def schedule_norm(hp, qb, opss):
            # normalization pipeline: row HD of ops is the softmax
            # denominator.  All reads run straight off the AV PSUM (the
            # reciprocal's rows 0-63 are junk but free: DVE time depends on
            # the free dim only); the PSUM set frees at s_mul, two units
            # before its next use.  Stage emissions are spread over
            # subsequent PE steps (~0.6us each) so every op's inputs are
            # ready before its engine reaches it.
            cols = slice(qb * QB, (qb + 1) * QB)
            rec0 = nrm.tile([VW, QB], F32, tag="rec0", name="rec0")
            rec1 = nrm.tile([VW, QB], F32, tag="rec1", name="rec1")
            den0 = nrm.tile([1, QB], F32, tag="den0", name="den0")
            den1 = nrm.tile([1, QB], F32, tag="den1", name="den1")
            bc0 = nrm.tile([HD, QB], F32, tag="bc0", name="bc0")
            bc1 = nrm.tile([HD, QB], F32, tag="bc1", name="bc1")
            odd = nrm.tile([HD, QB], MMDT, tag="odd", name="odd")

            def s_recip():
                nc.vector.reciprocal_approx_fast(out=rec0, in_=opss[0][0:VW, :])
                nc.vector.reciprocal_approx_fast(out=rec1, in_=opss[1][0:VW, :])
                nc.sync.dma_start(out=den0, in_=rec0[HD : HD + 1, :])
                nc.sync.dma_start(out=den1, in_=rec1[HD : HD + 1, :])

            def s_bcast():
                nc.gpsimd.partition_broadcast(bc0, den0)
                nc.gpsimd.partition_broadcast(bc1, den1)

            def s_mul():
                nc.vector.tensor_mul(attn_p[hp][0:HD, cols], opss[0][0:HD, :], bc0)
                # odd head: normalize into a scratch at lanes 0-63, then
                # DMA-hop to lanes 64-127 of the pair tile
                nc.vector.tensor_mul(odd, opss[1][0:HD, :], bc1)
                nc.sync.dma_start(out=attn_p[hp][HD:P, cols], in_=odd)

            defer(2, s_recip)
            defer(5, s_bcast)
            defer(7, s_mul)
            if hp == 1:
                for k, tt in enumerate(range(qb * (QB // P), (qb + 1) * (QB // P))):
                    defer(11 + k, lambda tt=tt: emit_outproj_tt(tt))

        units = [(qb, hp) for hp in range(HPC // 2) for qb in range(NU)]
        pending = None  # av_emitter from the previous step
        for u, (qb, hp) in enumerate(units):
            pair = (2 * hp, 2 * hp + 1)
            qhs = [
                qT_sb[(h % 2) * HD : (h % 2) * HD + HD, h // 2, :] for h in pair
            ]
            khs = [
                kT_sb[(h % 2) * HD : (h % 2) * HD + HD, h // 2, :] for h in pair
            ]
            st = "ab"[u % 2]
            opss = [
                aps.tile([P, QB], F32, tag=f"o{i}{st}", name=f"o{i}{st}")
                for i in range(2)
            ]
            nkt = (qb + 1) * (QB // P)
            for kt in range(nkt):
                off = max(0, kt * P - qb * QB)
                # scores for both heads into one [128, 2, 512] psum tile
                sps = mmps.tile([P, 2, QB], F32, tag="mm", name="sps")
                for i in (0, 1):
                    nc.tensor.matmul(
                        sps[:, i, off:QB],
                        lhsT=khs[i][:, kt * P : (kt + 1) * P],
                        rhs=qhs[i][:, qb * QB + off : (qb + 1) * QB],
                        start=True,
                        stop=True,
                    )
                eT = etp.tile([P, 2, QB], MMDT, tag="eT", name="eT")
                if kt % 7 in (1, 3, 6):  # ~43% of tiles on DVE fast-exp
                    nc.vector.tensor_scalar(
                        out=eT[:, :, off:QB].bitcast(I16),
                        in0=sps[:, :, off:QB],
                        scalar1=FE_A, scalar2=FE_B,
                        op0=mybir.AluOpType.mult,
                        op1=mybir.AluOpType.add,
                    )
                else:
                    nc.scalar.activation(
                        out=eT[:, :, off:QB], in_=sps[:, :, off:QB],
                        func=EXP, scale=1.0 / np.sqrt(HD),
                    )
                if kt * P >= qb * QB:  # diagonal-crossing block (gpsimd is
                    # safe for the odd head: all its other queued ops are
                    # deferred-emitted, so nothing ahead of the mask waits)
                    nc.vector.tensor_mul(
                        eT[:, 0, off : off + P], eT[:, 0, off : off + P], triu
                    )
                    nc.gpsimd.tensor_mul(
                        eT[:, 1, off : off + P], eT[:, 1, off : off + P], triu
                    )

                # emit the PREVIOUS step's AV matmuls now (one-step lag keeps
                # the in-order PE queue from stalling on this tile's exp),
                # then any due deferred work
                if pending is not None:
                    pending()
                flush()

                def av(eT=eT, off=off, kt=kt, opss=opss, pair=pair, nkt=nkt,
                       hp=hp, qb=qb):
                    for i in (0, 1):
                        nc.tensor.matmul(
                            opss[i][0:VW, off:QB],
                            lhsT=v_sb[:, kt, pair[i] * VW : (pair[i] + 1) * VW],
                            rhs=eT[:, i, off:QB],
                            start=(kt == 0),
                            stop=(kt == nkt - 1),
                        )
                    if kt == nkt - 1:
                        schedule_norm(hp, qb, opss)

                pending = av
                step += 1
        pending()
        while deferred:
            heapq.heappop(deferred)[2]()

    nc.compile()
    return nc


def make_in_maps(x, W_qkv, b_qkv, W_out):
    x = np.asarray(x, np.float32)
    W_qkv = np.asarray(W_qkv, np.float32)
    b_qkv = np.asarray(b_qkv, np.float32)
    W_out = np.asarray(W_out, np.float32)
    in_maps = []
    for c in range(2 * B):
        b, g = divmod(c, 2)
        ch = g * HPC * HD
        wqa = W_qkv[:, ch : ch + 256].reshape(KT, P, 2 * P).transpose(1, 0, 2)
        wka = W_qkv[:, D + ch : D + ch + 256].reshape(KT, P, 2 * P).transpose(1, 0, 2)
        wqkb = np.concatenate(
            [
                b_qkv[ch : ch + 256].reshape(2, P).T,
                b_qkv[D + ch : D + ch + 256].reshape(2, P).T,
            ],
            axis=1,
        )  # [128, 4]: cols = q-m0, q-m1, k-m0, k-m1
        wva = np.zeros((D, HPC * VW), np.float32)
        wva3 = wva.reshape(D, HPC, VW)
        wva3[:, :, :HD] = W_qkv[:, 2 * D + ch : 2 * D + ch + 256].reshape(D, HPC, HD)
        wva = wva.reshape(KT, P, HPC * VW).transpose(1, 0, 2)
        wo = W_out[ch : ch + 256, :].reshape(2, P, D).transpose(1, 0, 2)
        in_maps.append(
            {
                "xT": np.ascontiguousarray(x[b].T).astype(_NP_MMDT),
                "wqa": np.ascontiguousarray(wqa).astype(_NP_MMDT),
                "wka": np.ascontiguousarray(wka).astype(_NP_MMDT),
                "wva": np.ascontiguousarray(wva).astype(_NP_MMDT),
                "wqkb": np.ascontiguousarray(wqkb, np.float32),
                "wo": np.ascontiguousarray(wo).astype(_NP_MMDT),
            }
        )
    return in_maps


def assemble(results, b_out, vbias_y):
    b_out = np.asarray(b_out, np.float32) + vbias_y
    out = np.empty((B, T, D), np.float32)
    for b in range(B):
        out[b] = (
            results[2 * b]["y"].astype(np.float32)
            + results[2 * b + 1]["y"].astype(np.float32)
            + b_out[None, :]
        )
    return out


_CACHE = {}


def kernel(x, W_qkv, b_qkv, W_out, b_out):
    if "nc" not in _CACHE:
        _CACHE["nc"] = build_bass()
    in_maps = make_in_maps(x, W_qkv, b_qkv, W_out)
    # v-bias contribution: softmax weights sum to 1, so b_v passes through
    # attention unchanged and lands as (b_v @ W_out) on every token.
    vbias_y = np.asarray(b_qkv, np.float32)[2 * D :] @ np.asarray(W_out, np.float32)
    res = run_bass_kernel_spmd(_CACHE["nc"], in_maps, list(range(2 * B)))
    return assemble(res.results, b_out, vbias_y)


# revision 20
# speedup vs baseline: 1.8427x; 1.4082x over previous
"""Multi-head causal self-attention on 8 Trainium2 NeuronCores.

Sharding: core c -> batch b = c // 2, heads 4*(c % 2) .. +4  (data parallel on
B, tensor parallel on heads).  Each core computes its 4 heads' attention for
its batch plus the partial out-projection; the host sums the two partials per
batch and adds b_out.

Per-core layout:
  xT   [D, T]      x[b] transposed on host (bf16)
  qT/kT [128, 2, T] head-major: partitions = 2 heads x 64, 2 m-tiles
  v    [128, 16, 260] natural [T, hd] per head + a ones column (gives the
                    softmax denominator for free during the AV matmul)
  scores are computed transposed: sT[k, q] = kT.T @ q, both heads of a pair
  packed into one [128, 2, 512] PSUM tile so a single evacuation call
  covers both; exp'd during PSUM->SBUF evacuation (no max subtraction:
  |scores| <= ~3 here), causality via tile skipping/trimming + one
  upper-triangular 128x128 mask constant.

Attention runs in 512-query units (qb, head-pair).  The PE instruction
stream is software-pipelined with the AV matmuls lagging the score
matmuls by one key-tile step -- the PE queue is strictly in-order, so
without the lag every AV would stall on its own tile's exp evacuation.
AV accumulator PSUM sets alternate per unit so normalization (which
starts with an immediate PSUM-freeing copy) overlaps the next units.
The exp evacuation alternates by key-tile parity between ACT (exact
spline exp) and DVE (Schraudolph bit-trick exp2 via tensor_scalar ->
int16 -> bf16 bitcast, ~3% per element, cancels in softmax).  The
out-projection packs head pairs into K=128 accumulations (odd heads hop
partitions via SBUF-to-SBUF DMA).
"""

import os
import sys
from contextlib import ExitStack

import numpy as np

for _p in ("/opt/trn_rl_repo", "/opt/pypackages"):
    if os.path.isdir(_p) and _p not in sys.path:
        sys.path.append(_p)

import concourse.bass as bass
from concourse import bacc
import concourse.mybir as mybir
import concourse.tile as tile
from concourse.bass_utils import run_bass_kernel_spmd
from concourse.masks import make_upper_triangular


B, T, D = 4, 2048, 512
H, HD = 8, 64
HPC = 4  # heads per core
P = 128
KT = D // P  # k-tiles over the model dim
QB = 512  # query-unit width / psum bank width
NKT = T // P  # key tiles
NU = T // QB  # query blocks
VW = HD + 1  # v columns per head incl. the ones column

F32 = mybir.dt.float32
I16 = mybir.dt.int16
BF16 = mybir.dt.bfloat16
MMDT = BF16
EXP = mybir.ActivationFunctionType.Exp

# fast-exp constants: exp(s/8) ~= bitcast_bf16(int16(s * FE_A + FE_B))
_LOG2E = 1.4426950408889634
FE_A = _LOG2E * 128.0 / 8.0
FE_B = 127.0 * 128.0 - 5.6

try:
    import ml_dtypes
    _NP_MMDT = np.dtype(ml_dtypes.bfloat16)
except ImportError:
    _NP_MMDT = np.float32


def build_bass():
    nc = bacc.Bacc()
    xT = nc.declare_dram_parameter("xT", [D, T], MMDT, isOutput=False)
    wqa = nc.declare_dram_parameter("wqa", [P, KT, 2 * P], MMDT, isOutput=False)
    wka = nc.declare_dram_parameter("wka", [P, KT, 2 * P], MMDT, isOutput=False)
    # q/k biases, laid out [channel % 128, channel // 128] for ACT bias APs
    wqkb = nc.declare_dram_parameter("wqkb", [P, 4], F32, isOutput=False)
    wva = nc.declare_dram_parameter("wva", [P, KT, HPC * VW], MMDT, isOutput=False)
    wo = nc.declare_dram_parameter("wo", [P, 2, D], MMDT, isOutput=False)
    y = nc.declare_dram_parameter("y", [T, D], BF16, isOutput=True)

    with tile.TileContext(nc) as tc, ExitStack() as ctx:
        consts = ctx.enter_context(tc.tile_pool(name="consts", bufs=1))
        qkv = ctx.enter_context(tc.tile_pool(name="qkv", bufs=1))
        attn = ctx.enter_context(tc.tile_pool(name="attn", bufs=1))
        etp = ctx.enter_context(tc.tile_pool(name="etp", bufs=4))
        nrm = ctx.enter_context(tc.tile_pool(name="nrm", bufs=3))
        yevac = ctx.enter_context(tc.tile_pool(name="yevac", bufs=3))
        # PSUM: "mm" 2 bufs x 4KB/partition (2 banks each) = 4 banks;
        # o{i}{s} 4 x [128,512]f32 (1 bank each) = 4 banks.  AV accumulator
        # sets s alternate per unit; QKV + out-proj borrow "mm".
        mmps = ctx.enter_context(tc.tile_pool(name="mmps", bufs=2, space="PSUM"))
        aps = ctx.enter_context(tc.tile_pool(name="aps", bufs=1, space="PSUM"))

        # ---- inputs / constants into SBUF (ordered for early compute start;
        # issue spread across the three DMA-capable engine queues)
        x_sb = consts.tile([P, KT, T], MMDT)
        wq_sb = consts.tile([P, KT, 2 * P], MMDT)
        wk_sb = consts.tile([P, KT, 2 * P], MMDT)
        wv_sb = consts.tile([P, KT, HPC * VW], MMDT)
        wqkb_sb = consts.tile([P, 4], F32)
        wo_sb = consts.tile([P, 2, D], MMDT)

        nc.sync.dma_start(out=wq_sb, in_=wqa[:])
        nc.scalar.dma_start(out=wk_sb, in_=wka[:])
        for kt in range(KT):
            eng = (nc.sync, nc.scalar, nc.gpsimd, nc.sync)[kt]
            eng.dma_start(
                out=x_sb[:, kt, 0 : T // 2], in_=xT[kt * P : (kt + 1) * P, 0 : T // 2]
            )
        nc.gpsimd.dma_start(out=wqkb_sb, in_=wqkb[:])
        nc.scalar.dma_start(out=wv_sb, in_=wva[:])
        for kt in range(KT):
            eng = (nc.sync, nc.scalar, nc.gpsimd, nc.sync)[kt]
            eng.dma_start(
                out=x_sb[:, kt, T // 2 : T], in_=xT[kt * P : (kt + 1) * P, T // 2 : T]
            )
        nc.gpsimd.dma_start(out=wo_sb, in_=wo[:])

        # triu[k, q] = 1 iff q >= k: allowed region of a diagonal block in
        # transposed-score space.  gpsimd affine_select needs f32; cast after.
        triu_st = consts.tile([P, P], F32)
        make_upper_triangular(nc, triu_st, val=1.0, diag=True)
        triu = consts.tile([P, P], MMDT)
        nc.vector.tensor_copy(triu, triu_st)

        # ---- QKV projections
        qT_sb = qkv.tile([P, 2, T], MMDT)
        kT_sb = qkv.tile([P, 2, T], MMDT)
        v_sb = qkv.tile([P, NKT, HPC * VW], MMDT)

        WB = 1024  # bf16 moving-operand max

        def qk_proj(wi, w_sb, dst, m, nb):
            ps = mmps.tile([P, WB], F32, tag="mm", name="ps")
            for lo in range(0, WB, QB):  # psum-bank-sized writes
                for kt in range(KT):
                    nc.tensor.matmul(
                        ps[:, lo : lo + QB],
                        lhsT=w_sb[:, kt, m * P : (m + 1) * P],
                        rhs=x_sb[:, kt, nb * WB + lo : nb * WB + lo + QB],
                        start=(kt == 0),
                        stop=(kt == KT - 1),
                    )
            nc.scalar.activation(
                out=dst[:, m, nb * WB : (nb + 1) * WB], in_=ps,
                func=mybir.ActivationFunctionType.Identity,
                bias=wqkb_sb[:, 2 * wi + m : 2 * wi + m + 1],
            )

        # v bias is folded into the host-side output bias (b_v @ W_out adds a
        # constant row after softmax-normalize + out-projection), so v here is
        # bias-free; the denominator ones-columns are memset directly.
        def v_proj(tt):
            tag = f"o{tt % 2}{'ab'[(tt // 2) % 2]}"
            ps = aps.tile([P, QB], F32, tag=tag, name="vps")
            for kt in range(KT):
                nc.tensor.matmul(
                    ps[:, 0 : HPC * VW],
                    lhsT=x_sb[:, kt, tt * P : (tt + 1) * P],
                    rhs=wv_sb[:, kt, :],
                    start=(kt == 0),
                    stop=(kt == KT - 1),
                )
            nc.scalar.copy(v_sb[:, tt, :], ps[:, 0 : HPC * VW])
            ones_cols = v_sb[:, tt, :].rearrange("p (h w) -> p h w", w=VW)[:, :, HD]
            nc.gpsimd.memset(ones_cols, 1.0)

        # issue order: everything the first attention units need first
        for m in range(2):
            qk_proj(0, wq_sb, qT_sb, m, 0)
            qk_proj(1, wk_sb, kT_sb, m, 0)
        for tt in range(NKT // 2):
            v_proj(tt)
        for m in range(2):
            qk_proj(0, wq_sb, qT_sb, m, 1)
            qk_proj(1, wk_sb, kT_sb, m, 1)
        for tt in range(NKT // 2, NKT):
            v_proj(tt)

        # ---- attention in 512-query units, PE-stream software-pipelined
        # attn pair tiles: rows 0-63 = even head, 64-127 = odd head of pair
        attn_p = [
            attn.tile([P, T], MMDT, tag=f"attnp{hp}", name=f"attnp{hp}")
            for hp in range(2)
        ]

        def emit_norm(hp, qb, opss):
            # normalize: row HD of ops is the softmax denominator.  The
            # copy evacuates the AV psum immediately (frees the bank); the
            # rest of the chain runs off SBUF.
            cols = slice(qb * QB, (qb + 1) * QB)
            for i in (0, 1):
                att_sb = nrm.tile([VW, QB], F32, tag="att", name="att")
                if i == 0:
                    nc.vector.tensor_copy(att_sb, opss[i][0:VW, :])
                else:
                    nc.scalar.copy(att_sb, opss[i][0:VW, :])
                rec = nrm.tile([VW, QB], F32, tag="rec", name="rec")
                nc.vector.reciprocal_approx_fast(out=rec, in_=att_sb)
                den0 = nrm.tile([1, QB], F32, tag="den0", name="den0")
                nc.sync.dma_start(out=den0, in_=rec[HD : HD + 1, :])
                bc = nrm.tile([HD, QB], F32, tag="bc", name="bc")
                nc.gpsimd.partition_broadcast(bc, den0)
                if i == 0:
                    nc.vector.tensor_mul(
                        attn_p[hp][0:HD, cols], att_sb[0:HD, :], bc
                    )
                else:
                    # odd head: normalize into a scratch at lanes 0-63,
                    # then DMA-hop to lanes 64-127 of the pair tile
                    odd = nrm.tile([HD, QB], MMDT, tag="odd", name="odd")
                    nc.vector.tensor_mul(odd, att_sb[0:HD, :], bc)
                    nc.sync.dma_start(out=attn_p[hp][HD:P, cols], in_=odd)

        units = [(qb, hp) for hp in range(HPC // 2) for qb in range(NU)]
        pending = None  # (av_emitter, is_last_kt, norm_emitter)
        for u, (qb, hp) in enumerate(units):
            pair = (2 * hp, 2 * hp + 1)
            qhs = [
                qT_sb[(h % 2) * HD : (h % 2) * HD + HD, h // 2, :] for h in pair
            ]
            khs = [
                kT_sb[(h % 2) * HD : (h % 2) * HD + HD, h // 2, :] for h in pair
            ]
            st = "ab"[u % 2]
            opss = [
                aps.tile([P, QB], F32, tag=f"o{i}{st}", name=f"o{i}{st}")
                for i in range(2)
            ]
            nkt = (qb + 1) * (QB // P)
            for kt in range(nkt):
                off = max(0, kt * P - qb * QB)
                # scores for both heads into one [128, 2, 512] psum tile
                sps = mmps.tile([P, 2, QB], F32, tag="mm", name="sps")
                for i in (0, 1):
                    nc.tensor.matmul(
                        sps[:, i, off:QB],
                        lhsT=khs[i][:, kt * P : (kt + 1) * P],
                        rhs=qhs[i][:, qb * QB + off : (qb + 1) * QB],
                        start=True,
                        stop=True,
                    )
                eT = etp.tile([P, 2, QB], MMDT, tag="eT", name="eT")
                if kt % 2:  # DVE fast-exp on odd key tiles
                    nc.vector.tensor_scalar(
                        out=eT[:, :, off:QB].bitcast(I16),
                        in0=sps[:, :, off:QB],
                        scalar1=FE_A, scalar2=FE_B,
                        op0=mybir.AluOpType.mult,
                        op1=mybir.AluOpType.add,
                    )
                else:
                    nc.scalar.activation(
                        out=eT[:, :, off:QB], in_=sps[:, :, off:QB],
                        func=EXP, scale=1.0 / np.sqrt(HD),
                    )
                if kt * P >= qb * QB:  # diagonal-crossing block
                    for i in (0, 1):
                        nc.vector.tensor_mul(
                            eT[:, i, off : off + P], eT[:, i, off : off + P], triu
                        )

                # emit the PREVIOUS step's AV matmuls now (one-step lag keeps
                # the in-order PE queue from stalling on this tile's exp)
                if pending is not None:
                    pending[0]()
                    if pending[1]:
                        pending[2]()

                def av(eT=eT, off=off, kt=kt, opss=opss, pair=pair, nkt=nkt):
                    for i in (0, 1):
                        nc.tensor.matmul(
                            opss[i][0:VW, off:QB],
                            lhsT=v_sb[:, kt, pair[i] * VW : (pair[i] + 1) * VW],
                            rhs=eT[:, i, off:QB],
                            start=(kt == 0),
                            stop=(kt == nkt - 1),
                        )

                def norm(hp=hp, qb=qb, opss=opss):
                    emit_norm(hp, qb, opss)

                pending = (av, kt == nkt - 1, norm)
        pending[0]()
        pending[2]()

        # ---- out projection:
        # y[t, d] = attn_p0.T @ wo_p0 + attn_p1.T @ wo_p1  (K=128 each)
        for tt in range(NKT):
            ps = mmps.tile([P, D], F32, tag="mm", name="yps")
            for hp in range(2):
                nc.tensor.matmul(
                    ps,
                    lhsT=attn_p[hp][:, tt * P : (tt + 1) * P],
                    rhs=wo_sb[:, hp, :],
                    start=(hp == 0),
                    stop=(hp == 1),
                )
            yt = yevac.tile([P, D], BF16, tag="yt", name="yt")
            if tt % 2:
                nc.scalar.copy(yt, ps)
            else:
                nc.vector.tensor_copy(yt, ps)
            nc.sync.dma_start(out=y[tt * P : (tt + 1) * P, :], in_=yt)

    nc.compile()
    return nc


def make_in_maps(x, W_qkv, b_qkv, W_out):
    x = np.asarray(x, np.float32)
    W_qkv = np.asarray(W_qkv, np.float32)
    b_qkv = np.asarray(b_qkv, np.float32)
    W_out = np.asarray(W_out, np.float32)
    in_maps = []
    for c in range(2 * B):
        b, g = divmod(c, 2)
        ch = g * HPC * HD
        wqa = W_qkv[:, ch : ch + 256].reshape(KT, P, 2 * P).transpose(1, 0, 2)
        wka = W_qkv[:, D + ch : D + ch + 256].reshape(KT, P, 2 * P).transpose(1, 0, 2)
        wqkb = np.concatenate(
            [
                b_qkv[ch : ch + 256].reshape(2, P).T,
                b_qkv[D + ch : D + ch + 256].reshape(2, P).T,
            ],
            axis=1,
        )  # [128, 4]: cols = q-m0, q-m1, k-m0, k-m1
        wva = np.zeros((D, HPC * VW), np.float32)
        wva3 = wva.reshape(D, HPC, VW)
        wva3[:, :, :HD] = W_qkv[:, 2 * D + ch : 2 * D + ch + 256].reshape(D, HPC, HD)
        wva = wva.reshape(KT, P, HPC * VW).transpose(1, 0, 2)
        wo = W_out[ch : ch + 256, :].reshape(2, P, D).transpose(1, 0, 2)
        in_maps.append(
            {
                "xT": np.ascontiguousarray(x[b].T).astype(_NP_MMDT),
                "wqa": np.ascontiguousarray(wqa).astype(_NP_MMDT),
                "wka": np.ascontiguousarray(wka).astype(_NP_MMDT),
                "wva": np.ascontiguousarray(wva).astype(_NP_MMDT),
                "wqkb": np.ascontiguousarray(wqkb, np.float32),
                "wo": np.ascontiguousarray(wo).astype(_NP_MMDT),
            }
        )
    return in_maps


def assemble(results, b_out, vbias_y):
    b_out = np.asarray(b_out, np.float32) + vbias_y
    out = np.empty((B, T, D), np.float32)
    for b in range(B):
        out[b] = (
            results[2 * b]["y"].astype(np.float32)
            + results[2 * b + 1]["y"].astype(np.float32)
            + b_out[None, :]
        )
    return out


_CACHE = {}


def kernel(x, W_qkv, b_qkv, W_out, b_out):
    if "nc" not in _CACHE:
        _CACHE["nc"] = build_bass()
    in_maps = make_in_maps(x, W_qkv, b_qkv, W_out)
    # v-bias contribution: softmax weights sum to 1, so b_v passes through
    # attention unchanged and lands as (b_v @ W_out) on every token.
    vbias_y = np.asarray(b_qkv, np.float32)[2 * D :] @ np.asarray(W_out, np.float32)
    res = run_bass_kernel_spmd(_CACHE["nc"], in_maps, list(range(2 * B)))
    return assemble(res.results, b_out, vbias_y)
